# revision 2
# baseline (speedup 1.0000x reference)
"""Trainium2 Bass kernel: multi-head elementwise-attention GNN message passing.

Full inputs -> full output. Internally: edges partitioned by destination-node
block across 8 NeuronCores; k/v projections replicated; per-edge gathers via
indirect DMA; segment sums via one-hot matmuls accumulated in PSUM.
"""
import sys
sys.path.insert(0, '/opt/trn_rl_repo')
import math
import numpy as np
import ml_dtypes

import concourse.bass as bass
import concourse.bacc as bacc
import concourse.mybir as mybir
import concourse.tile as tile
from concourse import bass2jax

P = 128
D = 128
N_CORES = 8
KV_BF16 = True

_cache = {}


def _build(nblk_core, t_b, n_all_blk, kv_bf16=KV_BF16):
    """Build+compile the per-core Bass module.

    nblk_core: node blocks owned by each core (output range)
    t_b:       tiles (128 edges each) per block, fixed
    n_all_blk: total node blocks (padded N / 128), projections replicated
    """
    key = (nblk_core, t_b, n_all_blk, kv_bf16)
    if key in _cache:
        return _cache[key]
    n_pad = n_all_blk * P
    n_core = nblk_core * P
    ncols = nblk_core * t_b
    kv_dt = mybir.dt.bfloat16 if kv_bf16 else mybir.dt.float32
    f32 = mybir.dt.float32

    nc = bacc.Bacc("TRN2", target_bir_lowering=False, debug=False,
                   num_devices=N_CORES)
    # ---- I/O ----
    xT = nc.dram_tensor("xT", [P, n_pad], f32, kind="ExternalInput")
    xTq = nc.dram_tensor("xTq", [P, n_core], f32, kind="ExternalInput")
    wk = nc.dram_tensor("wk", [D, D], f32, kind="ExternalInput")
    wv = nc.dram_tensor("wv", [D, D], f32, kind="ExternalInput")
    wq = nc.dram_tensor("wq", [D, D], f32, kind="ExternalInput")
    wo = nc.dram_tensor("wo", [D, D], f32, kind="ExternalInput")
    bkv = nc.dram_tensor("bkv", [P, 2 * D], f32, kind="ExternalInput")
    bq = nc.dram_tensor("bq", [P, D], f32, kind="ExternalInput")
    iotaF = nc.dram_tensor("iotaF", [P, P], f32, kind="ExternalInput")
    iotaP = nc.dram_tensor("iotaP", [P, 1], f32, kind="ExternalInput")
    ones1 = nc.dram_tensor("ones1", [1, P], f32, kind="ExternalInput")
    srcoff = nc.dram_tensor("srcoff", [P, ncols], mybir.dt.int32,
                            kind="ExternalInput")
    offc = nc.dram_tensor("offc", [P, ncols], f32, kind="ExternalInput")
    offr = nc.dram_tensor("offr", [1, ncols * P], f32, kind="ExternalInput")
    outT = nc.dram_tensor("outT", [P, n_core], f32, kind="ExternalOutput")

    with tile.TileContext(nc) as tc:
        with tc.tile_pool(name="const", bufs=1) as cp, \
             tc.tile_pool(name="qres", bufs=1) as qp, \
             tc.tile_pool(name="dram", bufs=1, space="DRAM") as dp, \
             tc.tile_pool(name="xld", bufs=4) as xp, \
             tc.tile_pool(name="kvw", bufs=4) as kp, \
             tc.tile_pool(name="meta", bufs=3) as mp, \
             tc.tile_pool(name="gath", bufs=12) as gp, \
             tc.tile_pool(name="work", bufs=4) as wp, \
             tc.tile_pool(name="epi", bufs=3) as ep, \
             tc.tile_pool(name="ps", bufs=6, space="PSUM") as pp, \
             tc.tile_pool(name="psz", bufs=1, space="PSUM") as pz:

            # ---- constants to SBUF ----
            wk_s = cp.tile([D, D], f32); nc.sync.dma_start(out=wk_s[:], in_=wk.ap())
            wv_s = cp.tile([D, D], f32); nc.sync.dma_start(out=wv_s[:], in_=wv.ap())
            wq_s = cp.tile([D, D], f32); nc.sync.dma_start(out=wq_s[:], in_=wq.ap())
            wo_s = cp.tile([D, D], f32); nc.sync.dma_start(out=wo_s[:], in_=wo.ap())
            bkv_s = cp.tile([P, 2 * D], f32); nc.sync.dma_start(out=bkv_s[:], in_=bkv.ap())
            bq_s = cp.tile([P, D], f32); nc.sync.dma_start(out=bq_s[:], in_=bq.ap())
            iF_s = cp.tile([P, P], f32); nc.sync.dma_start(out=iF_s[:], in_=iotaF.ap())
            iP_s = cp.tile([P, 1], f32); nc.sync.dma_start(out=iP_s[:], in_=iotaP.ap())
            on_s = cp.tile([1, P], f32); nc.sync.dma_start(out=on_s[:], in_=ones1.ap())
            zb_s = cp.tile([P, 1], f32); nc.vector.memset(zb_s[:], 0.0)

            kv_dram = dp.tile([n_pad, 2 * D], kv_dt)

            # ---- Phase A: kv = [x@Wk+bk | x@Wv+bv] for ALL nodes ----
            for b in range(n_all_blk):
                xt = xp.tile([P, P], f32, tag="xt")
                nc.sync.dma_start(out=xt[:], in_=xT.ap()[:, b * P:(b + 1) * P])
                pkv = pp.tile([P, 2 * D], f32, tag="mm")
                nc.tensor.matmul(out=pkv[:, 0:D], lhsT=xt[:], rhs=wk_s[:],
                                 start=True, stop=True)
                nc.tensor.matmul(out=pkv[:, D:2 * D], lhsT=xt[:], rhs=wv_s[:],
                                 start=True, stop=True)
                kv_t = kp.tile([P, 2 * D], kv_dt, tag="kvw")
                nc.vector.tensor_tensor(out=kv_t[:], in0=pkv[:], in1=bkv_s[:],
                                        op=mybir.AluOpType.add)
                nc.sync.dma_start(out=kv_dram[b * P:(b + 1) * P, :], in_=kv_t[:])

            # ---- Phase B: q for this core's blocks, kept in SBUF ----
            q_s = qp.tile([P, n_core], f32)
            for j in range(nblk_core):
                xt = xp.tile([P, P], f32, tag="xt")
                nc.sync.dma_start(out=xt[:], in_=xTq.ap()[:, j * P:(j + 1) * P])
                pq = pp.tile([P, 2 * D], f32, tag="mm")
                nc.tensor.matmul(out=pq[:, 0:D], lhsT=xt[:], rhs=wq_s[:],
                                 start=True, stop=True)
                nc.vector.tensor_tensor(out=q_s[:, j * P:(j + 1) * P], in0=pq[:, 0:D],
                                        in1=bq_s[:], op=mybir.AluOpType.add)

            # ---- Phase C: per-block edge processing ----
            inv_sqrt_dk = 1.0 / math.sqrt(D // 8)  # d_k = 16
            for j in range(nblk_core):
                so_t = mp.tile([P, t_b], mybir.dt.int32, tag="so")
                nc.sync.dma_start(out=so_t[:], in_=srcoff.ap()[:, j * t_b:(j + 1) * t_b])
                oc_t = mp.tile([P, t_b], f32, tag="oc")
                nc.sync.dma_start(out=oc_t[:], in_=offc.ap()[:, j * t_b:(j + 1) * t_b])
                or_t = mp.tile([1, t_b * P], f32, tag="or")
                nc.sync.dma_start(out=or_t[:], in_=offr.ap()[:, j * t_b * P:(j + 1) * t_b * P])

                zT = pz.tile([P, P], f32, tag="zT")
                nT = pz.tile([P, P], f32, tag="nT")
                for t in range(t_b):
                    kv_t = gp.tile([P, 2 * D], kv_dt, tag="kv")
                    nc.gpsimd.indirect_dma_start(
                        out=kv_t[:], out_offset=None, in_=kv_dram[:],
                        in_offset=bass.IndirectOffsetOnAxis(ap=so_t[:, t:t + 1], axis=0))
                    # S_scatter[e,n] = (off[e]==n)
                    s_sc = wp.tile([P, P], f32, tag="ssc")
                    nc.vector.tensor_scalar(out=s_sc[:], in0=iF_s[:],
                                            scalar1=oc_t[:, t:t + 1], scalar2=None,
                                            op0=mybir.AluOpType.is_equal)
                    # S_gather[n,e] = (off[e]==n) via K=1 row broadcast
                    offb = pp.tile([P, 2 * D], f32, tag="mm")
                    nc.tensor.matmul(out=offb[:, 0:P], lhsT=on_s[:],
                                     rhs=or_t[:, t * P:(t + 1) * P],
                                     start=True, stop=True)
                    s_ga = wp.tile([P, P], f32, tag="sga")
                    nc.vector.tensor_tensor(out=s_ga[:], in0=offb[:, 0:P],
                                            in1=iP_s[:].to_broadcast([P, P]),
                                            op=mybir.AluOpType.is_equal)
                    # q_e = S_gather.T @ q_blk   [e, d]
                    qe = pp.tile([P, 2 * D], f32, tag="mm")
                    nc.tensor.matmul(out=qe[:, 0:D], lhsT=s_ga[:],
                                     rhs=q_s[:, j * P:(j + 1) * P],
                                     start=True, stop=True)
                    # m = exp(q_e * k_e / sqrt(d_k)); mv = m * v_e
                    t1 = wp.tile([P, D], f32, tag="t1")
                    nc.vector.tensor_tensor(out=t1[:], in0=qe[:, 0:D], in1=kv_t[:, 0:D],
                                            op=mybir.AluOpType.mult)
                    m_t = wp.tile([P, D], f32, tag="m")
                    nc.scalar.activation(m_t[:], t1[:],
                                         mybir.ActivationFunctionType.Exp,
                                         bias=zb_s[:], scale=inv_sqrt_dk)
                    mv_t = wp.tile([P, D], f32, tag="mv")
                    nc.vector.tensor_tensor(out=mv_t[:], in0=m_t[:],
                                            in1=kv_t[:, D:2 * D],
                                            op=mybir.AluOpType.mult)
                    # zT[d,n] += m.T @ S_sc ; nT[d,n] += mv.T @ S_sc
                    nc.tensor.matmul(out=zT[:], lhsT=m_t[:], rhs=s_sc[:],
                                     start=(t == 0), stop=(t == t_b - 1))
                    nc.tensor.matmul(out=nT[:], lhsT=mv_t[:], rhs=s_sc[:],
                                     start=(t == 0), stop=(t == t_b - 1))
                # epilogue: out_xT = nT / zT ; outT_blk = Wo.T-contract
                rz = ep.tile([P, P], f32, tag="rz")
                nc.vector.reciprocal(out=rz[:], in_=zT[:])
                ox = ep.tile([P, P], f32, tag="ox")
                nc.vector.tensor_tensor(out=ox[:], in0=nT[:], in1=rz[:],
                                        op=mybir.AluOpType.mult)
                po = pp.tile([P, 2 * D], f32, tag="mm")
                nc.tensor.matmul(out=po[:, 0:P], lhsT=wo_s[:], rhs=ox[:],
                                 start=True, stop=True)
                o_sb = ep.tile([P, P], f32, tag="osb")
                nc.vector.tensor_copy(out=o_sb[:], in_=po[:, 0:P])
                nc.sync.dma_start(out=outT.ap()[:, j * P:(j + 1) * P], in_=o_sb[:])

    nc.compile()
    _cache[key] = nc
    return nc


def kernel(x, src, dst, Wq, bq, Wk, bk, Wv, bv, Wo, bo):
    x = np.asarray(x, dtype=np.float32)
    n, d = x.shape
    assert d == D
    e = src.shape[0]
    src = np.asarray(src, dtype=np.int64)
    dst = np.asarray(dst, dtype=np.int64)

    n_all_blk = math.ceil(n / P)
    # pad total blocks to a multiple of N_CORES
    n_all_blk = math.ceil(n_all_blk / N_CORES) * N_CORES
    n_pad = n_all_blk * P
    nblk_core = n_all_blk // N_CORES
    n_core = nblk_core * P

    # ---- host prep: sort edges by dst block ----
    order = np.argsort(dst, kind="stable")
    sdst = dst[order].astype(np.int64)
    ssrc = src[order].astype(np.int64)
    blk = (sdst // P).astype(np.int64)
    counts = np.bincount(blk, minlength=n_all_blk)
    starts = np.zeros(n_all_blk + 1, dtype=np.int64)
    np.cumsum(counts, out=starts[1:])
    t_b = max(1, int(math.ceil(counts.max() / P)))

    ncols = nblk_core * t_b
    srcoff_np = np.zeros((N_CORES, P, ncols), dtype=np.int32)
    offc_np = np.full((N_CORES, P, ncols), 255.0, dtype=np.float32)
    for b in range(n_all_blk):
        c, j = divmod(b, nblk_core)
        s0, s1 = starts[b], starts[b + 1]
        cnt = s1 - s0
        if cnt == 0:
            continue
        cols = np.arange(cnt) // P + j * t_b
        rows = np.arange(cnt) % P
        srcoff_np[c, rows, cols] = ssrc[s0:s1]
        offc_np[c, rows, cols] = (sdst[s0:s1] - b * P).astype(np.float32)
    # offr: same values, row-major per tile [1, ncols*P]
    offr_np = np.ascontiguousarray(
        offc_np.transpose(0, 2, 1).reshape(N_CORES, 1, ncols * P))

    x_pad = np.zeros((n_pad, D), dtype=np.float32)
    x_pad[:n] = x
    xT_np = np.ascontiguousarray(x_pad.T)

    iotaF_np = np.tile(np.arange(P, dtype=np.float32)[None, :], (P, 1))
    iotaP_np = np.arange(P, dtype=np.float32)[:, None].copy()
    ones1_np = np.ones((1, P), dtype=np.float32)
    bkv_np = np.tile(np.concatenate([np.asarray(bk, np.float32),
                                     np.asarray(bv, np.float32)])[None, :], (P, 1))
    bq_np = np.tile(np.asarray(bq, np.float32)[None, :], (P, 1))

    nc = _build(nblk_core, t_b, n_all_blk)

    in_maps = []
    for c in range(N_CORES):
        in_maps.append({
            "xT": xT_np,
            "xTq": np.ascontiguousarray(xT_np[:, c * n_core:(c + 1) * n_core]),
            "wk": np.asarray(Wk, np.float32), "wv": np.asarray(Wv, np.float32),
            "wq": np.asarray(Wq, np.float32), "wo": np.asarray(Wo, np.float32),
            "bkv": bkv_np, "bq": bq_np,
            "iotaF": iotaF_np, "iotaP": iotaP_np, "ones1": ones1_np,
            "srcoff": srcoff_np[c], "offc": offc_np[c], "offr": offr_np[c],
        })
    results = bass2jax.run_bass_via_pjrt(nc, in_maps, n_cores=N_CORES)

    out = np.empty((n_pad, D), dtype=np.float32)
    for c in range(N_CORES):
        out[c * n_core:(c + 1) * n_core] = results[c]["outT"].T
    out = out[:n] + np.asarray(bo, np.float32)[None, :]
    return out.astype(np.float32)
